# revision 1
# baseline (speedup 1.0000x reference)
"""Focal-loss + smooth-L1 loss kernel for TRN2, SPMD over 8 NeuronCores.

Sharding: data-parallel over the batch axis (B=8 -> one batch row per core).
Each core computes, over its 76725 anchors:
  - per-class weighted histogram h[c]   = sum_n [lab_n==c] * (1-pt)^2 * (-logpt)
  - per-class count histogram  cnt[c]   = sum_n [lab_n==c]        (lab=-1 matches nothing)
  - per-partition smooth-L1 partial sums (positives-masked)
via a single streaming pass; the tiny final reduction happens on host.

Layout: anchor n = 600*p + t  (p = SBUF partition, t = free axis).
Tiles of T=75 anchors/partition; classes innermost, padded 81->82 for
even/aligned fp16 access. One-hot indicators (fp16) feed TensorE matmuls
(lhsT=iseq [P,82], rhs=[w0, 1] [P,2]) accumulating both histograms in PSUM.
"""

import numpy as np

import concourse.bass as bass
import concourse.bacc as bacc
import concourse.mybir as mybir
import concourse.tile as tile
from concourse.bass_utils import run_bass_kernel_spmd

F32 = mybir.dt.float32
F16 = mybir.dt.float16
I16 = mybir.dt.int16
AF = mybir.ActivationFunctionType
OP = mybir.AluOpType
AX = mybir.AxisListType

C = 81
CP = 82  # padded classes (even -> 4B-aligned fp16 rows; pad col never matches)


def build_kernel(A, APP, T):
    """A anchors, APP anchors/partition (p<127 full; p=127 has A-127*APP),
    T anchors/partition per tile. Requires APP % T == 0 and
    A - 127*APP == (APP//T - 1)*T  (i.e. last tile is the only partial-P one).
    """
    n_tiles = APP // T
    t_full = A - 127 * APP  # p=127 valid for t < t_full
    assert t_full == (n_tiles - 1) * T, (A, APP, T, t_full)

    nc = bacc.Bacc(None, target_bir_lowering=False)
    conf = nc.dram_tensor("conf", [A, C], F32, kind="ExternalInput")
    loc = nc.dram_tensor("loc", [A, 4], F32, kind="ExternalInput")
    tgt = nc.dram_tensor("tgt", [A, 5], F32, kind="ExternalInput")
    hist = nc.dram_tensor("hist", [C, 2], F32, kind="ExternalOutput")
    locs = nc.dram_tensor("locs", [128, 1], F32, kind="ExternalOutput")

    def dram_ap(h, row_elems, P, t0, width):
        # anchor n = APP*p + t ; element (n, f) at flat n*row_elems + f
        return bass.AP(
            tensor=h[:, :].tensor,
            offset=t0 * row_elems,
            ap=[[APP * row_elems, P], [row_elems, T], [1, width]],
        )

    with tile.TileContext(nc) as tc:
        with (
            tc.tile_pool(name="singles", bufs=1) as singles,
            tc.tile_pool(name="io", bufs=2) as io,
            tc.tile_pool(name="mid", bufs=2) as mid,
            tc.tile_pool(name="small", bufs=2) as small,
            tc.tile_pool(name="psum", bufs=1, space="PSUM") as psum,
        ):
            # constants
            iota_i = singles.tile([128, CP], I16)
            nc.gpsimd.iota(iota_i[:, :], [[1, CP]], channel_multiplier=0)
            iota16 = singles.tile([128, CP], F16)
            nc.vector.tensor_copy(iota16[:, :], iota_i[:, :])
            strip = singles.tile([128, n_tiles], F32)
            nc.vector.memset(strip[:, :], 0.0)

            ph = psum.tile([CP, 2], F32)

            for i in range(n_tiles):
                t0 = i * T
                P = 128 if i < n_tiles - 1 else 127

                conf_t = io.tile([128, T, C], F32, tag="conf")
                # split the big conf transfer across three engine DGE queues
                splits = [0, 43, 86, P]
                dma_engs = [nc.sync, nc.scalar, nc.gpsimd]
                for si in range(3):
                    p0, p1 = splits[si], splits[si + 1]
                    sub = bass.AP(
                        tensor=conf[:, :].tensor,
                        offset=p0 * APP * C + t0 * C,
                        ap=[[APP * C, p1 - p0], [C, T], [1, C]],
                    )
                    dma_engs[si].dma_start(conf_t[p0:p1], sub)
                tgt_t = io.tile([128, T, 5], F32, tag="tgt")
                nc.gpsimd.dma_start(tgt_t[:P], dram_ap(tgt, 5, P, t0, 5))
                loc_t = io.tile([128, T, 4], F32, tag="loc")
                nc.gpsimd.dma_start(loc_t[:P], dram_ap(loc, 4, P, t0, 4))

                tlab = tgt_t[:P, :, 4:5]  # [P,T,1] f32

                # ---- conf path ----
                e_t = mid.tile([128, T, CP], F16, tag="e")
                nc.gpsimd.memset(e_t[:P, :, C:CP], 0.0)
                nc.scalar.activation(e_t[:P, :, 0:C], conf_t[:P], AF.Exp)
                s_t = small.tile([128, T], F16, tag="s")
                with nc.allow_low_precision("fp32-internal accum; only final write is fp16"):
                    nc.vector.reduce_sum(s_t[:P], e_t[:P], axis=AX.X)

                iseq = mid.tile([128, T, CP], F16, tag="iseq")
                nc.vector.tensor_tensor(
                    iseq[:P],
                    iota16[:P, None, :].broadcast_to([P, T, CP]),
                    tlab.broadcast_to([P, T, CP]),
                    OP.is_equal,
                )
                me = mid.tile([128, T, CP], F16, tag="me")
                nc.vector.tensor_tensor(me[:P], iseq[:P], e_t[:P], OP.mult)
                ea = small.tile([128, T], F16, tag="ea")
                with nc.allow_low_precision("fp32-internal accum; only final write is fp16"):
                    nc.vector.reduce_sum(ea[:P], me[:P], axis=AX.X)

                # per-anchor scalars [P, T]
                eas = small.tile([128, T], F32, tag="eas")
                nc.vector.tensor_scalar_max(eas[:P], ea[:P], 1e-6)
                rs = small.tile([128, T], F32, tag="rs")
                nc.vector.reciprocal(rs[:P], s_t[:P])
                pt = small.tile([128, T], F32, tag="pt")
                nc.vector.tensor_tensor(pt[:P], ea[:P], rs[:P], OP.mult)
                pm1 = small.tile([128, T], F32, tag="pm1")
                nc.vector.tensor_scalar_add(pm1[:P], pt[:P], -1.0)
                usq = small.tile([128, T], F32, tag="usq")
                nc.scalar.activation(usq[:P], pm1[:P], AF.Square)
                lnea = small.tile([128, T], F32, tag="lnea")
                nc.scalar.activation(lnea[:P], eas[:P], AF.Ln)
                lns = small.tile([128, T], F32, tag="lns")
                nc.scalar.activation(lns[:P], s_t[:P], AF.Ln)
                nlp = small.tile([128, T], F32, tag="nlp")
                nc.vector.tensor_tensor(nlp[:P], lns[:P], lnea[:P], OP.subtract)

                wv = small.tile([128, T, 2], F16, tag="wv")
                nc.gpsimd.memset(wv[:P, :, 1:2], 1.0)
                nc.vector.tensor_tensor(wv[:P, :, 0:1], usq[:P, :, None], nlp[:P, :, None], OP.mult)

                for t in range(T):
                    nc.tensor.matmul(
                        ph[:, :],
                        iseq[:P, t, :],
                        wv[:P, t, :],
                        start=(i == 0 and t == 0),
                        stop=(i == n_tiles - 1 and t == T - 1),
                    )

                # ---- loc path ----
                df = small.tile([128, T, 4], F32, tag="df")
                nc.vector.tensor_tensor(df[:P], loc_t[:P], tgt_t[:P, :, 0:4], OP.subtract)
                da = small.tile([128, T, 4], F32, tag="da")
                nc.scalar.activation(da[:P], df[:P], AF.Abs)
                dm = small.tile([128, T, 4], F32, tag="dm")
                nc.vector.tensor_scalar_min(dm[:P], da[:P], 1.0)
                r_t = small.tile([128, T, 4], F32, tag="r")
                nc.vector.tensor_tensor(r_t[:P], da[:P], dm[:P], OP.subtract)
                sl1 = small.tile([128, T, 4], F32, tag="sl1")
                # (dm * 0.5*dm) + r  == 0.5*m^2 + (d - m)
                q_t = small.tile([128, T, 4], F32, tag="q")
                nc.vector.tensor_tensor(q_t[:P], dm[:P], dm[:P], OP.mult)
                nc.vector.scalar_tensor_tensor(sl1[:P], q_t[:P], 0.5, r_t[:P], OP.mult, OP.add)
                pos = small.tile([128, T], F16, tag="pos")
                nc.vector.tensor_scalar(pos[:P], tlab.squeeze(), 0.0, None, OP.is_gt)
                slm = small.tile([128, T, 4], F32, tag="slm")
                nc.vector.tensor_tensor(
                    slm[:P], sl1[:P], pos[:P, :, None].broadcast_to([P, T, 4]), OP.mult
                )
                nc.vector.reduce_sum(strip[:P, i : i + 1], slm[:P], axis=AX.XY)

            # ---- finalize ----
            hc = singles.tile([CP, 2], F32)
            nc.vector.tensor_copy(hc[:, :], ph[:, :])
            nc.sync.dma_start(hist[:, :], hc[0:C, :])
            lacc = singles.tile([128, 1], F32)
            nc.vector.reduce_sum(lacc[:, :], strip[:, :], axis=AX.X)
            nc.sync.dma_start(locs[:, :], lacc[:, :])

    nc.compile()
    return nc


_CACHED = {}


def _get_nc(A, APP, T):
    key = (A, APP, T)
    if key not in _CACHED:
        _CACHED[key] = build_kernel(A, APP, T)
    return _CACHED[key]


def combine_host(hists, locsums, alpha):
    """hists: [ncores, 81, 2]; locsums: [ncores, 128, 1]; alpha: [81]."""
    h = hists[:, :, 0].sum(axis=0).astype(np.float64)
    cnt = hists[:, :, 1].sum(axis=0).astype(np.float64)
    alpha = alpha.astype(np.float64)
    denom = np.clip(alpha * cnt, 1.0, None)
    conf_loss = np.sum(alpha * h / denom)
    num_pos = cnt[1:].sum()
    loc_sum = locsums.astype(np.float64).sum()
    denom_loc = max(num_pos * 4.0, 1.0)
    loc_loss = loc_sum / denom_loc if num_pos > 0 else 0.0
    return np.float32(loc_loss), np.float32(conf_loss)


def kernel(loc_pred, conf_pred, targets, alpha, _trace=False):
    B, A, _ = conf_pred.shape
    assert B == 8 and A == 76725
    nc = _get_nc(A, 600, 75)
    in_maps = [
        {
            "conf": np.ascontiguousarray(conf_pred[b], dtype=np.float32),
            "loc": np.ascontiguousarray(loc_pred[b], dtype=np.float32),
            "tgt": np.ascontiguousarray(targets[b], dtype=np.float32),
        }
        for b in range(B)
    ]
    res = run_bass_kernel_spmd(nc, in_maps, core_ids=list(range(B)), trace=_trace)
    hists = np.stack([r["hist"] for r in res.results])
    locsums = np.stack([r["locs"] for r in res.results])
    out = combine_host(hists, locsums, np.asarray(alpha, dtype=np.float32))
    if _trace:
        return out, res
    return out



# revision 11
# speedup vs baseline: 1.7403x; 1.7403x over previous
"""Focal-loss + smooth-L1 loss kernel for TRN2, SPMD over 8 NeuronCores.

Sharding: data-parallel over the batch axis (B=8 -> one batch row per core).
Each core computes, over its 76725 anchors:
  - per-class weighted histogram h[c]   = sum_n [lab_n==c] * (1-pt)^2 * (-logpt)
  - per-class count histogram  cnt[c]   = sum_n [lab_n==c]  (lab=-1 matches nothing)
  - per-partition smooth-L1 partial sums (positives-masked)
The tiny final reduction happens on host.

Inputs are cast to fp16 on host (halves HBM traffic; rel-err impact ~1e-3,
well under the 2e-2 gate). loc/boxes/labels ride one interleaved [A, 9]
tensor so each DMA descriptor is a contiguous 1350B chunk.

Layout: anchor n = 600*p + t (p = SBUF partition, t = free axis), tiles of
T=75 anchors/partition. All input DMAs are issued up-front into resident
SBUF tiles, spread over the three DGE queues (gpsimd SWDGE fans out across
all 16 DMA engines; sync/scalar HWDGE carry one tile each). Tiles are
processed in DMA-arrival order.

Per tile: exp on the scalar engine; row-sum s, logit gather la = conf[lab]
(one-hot multiply + reduce), and smooth-L1 partials on DVE; the one-hot
indicator matrix on gpsimd; per-class histograms via 75 PSUM-accumulated
TensorE matmuls (lhsT = iseq [P,82], rhs = [w, 1] [P,2]).
log pt = la - ln(s); w = (1-pt)^2 * (-log pt).
"""

import numpy as np

import concourse.bass as bass
import concourse.bacc as bacc
import concourse.mybir as mybir
import concourse.tile as tile
from concourse.bass_utils import run_bass_kernel_spmd

F32 = mybir.dt.float32
F16 = mybir.dt.float16
I16 = mybir.dt.int16
AF = mybir.ActivationFunctionType
OP = mybir.AluOpType
AX = mybir.AxisListType

C = 81
CP = 82  # padded classes (even -> aligned fp16 rows; pad col never matches)


def build_kernel(A, APP, T):
    """A anchors, APP anchors/partition, T anchors/partition per tile.
    Tiles 0..n-2 use all 128 partitions; the last tile covers only p<127
    (requires A - 127*APP == (APP//T - 1)*T)."""
    n_tiles = APP // T
    t_full = A - 127 * APP
    assert t_full == (n_tiles - 1) * T, (A, APP, T, t_full)

    nc = bacc.Bacc(None, target_bir_lowering=False)
    conf = nc.dram_tensor("conf", [A, C], F16, kind="ExternalInput")
    lt = nc.dram_tensor("lt", [A, 9], F16, kind="ExternalInput")
    hist = nc.dram_tensor("hist", [C, 2], F32, kind="ExternalOutput")
    locs = nc.dram_tensor("locs", [128, 1], F32, kind="ExternalOutput")

    def dram_ap(h, row_elems, P, t0):
        # anchor n = APP*p + t ; element (n, f) at flat n*row_elems + f
        return bass.AP(
            tensor=h[:, :].tensor,
            offset=t0 * row_elems,
            ap=[[APP * row_elems, P], [row_elems, T], [1, row_elems]],
        )

    proc_order = list(range(n_tiles))

    with tile.TileContext(nc) as tc:
        with (
            tc.tile_pool(name="singles", bufs=1) as singles,
            tc.tile_pool(name="io", bufs=3) as io,
            tc.tile_pool(name="epool", bufs=2) as epool,
            tc.tile_pool(name="ipool", bufs=2) as ipool,
            tc.tile_pool(name="mpool", bufs=2) as mpool,
            tc.tile_pool(name="small", bufs=2) as small,
            tc.tile_pool(name="psum", bufs=1, space="PSUM") as psum,
        ):
            # constants
            iota_i = singles.tile([128, CP], I16)
            nc.gpsimd.iota(iota_i[:, :], [[1, CP]], channel_multiplier=0)
            iota16 = singles.tile([128, CP], F16)
            nc.vector.tensor_copy(iota16[:, :], iota_i[:, :])
            strip = singles.tile([128, 2 * n_tiles], F32)
            nc.vector.memset(strip[:, :], 0.0)
            negone = singles.tile([128, 1], F32)
            nc.gpsimd.memset(negone[:, :], -1.0)

            ph = psum.tile([CP, 2], F32)

            def tp(i):
                return 128 if i < n_tiles - 1 else 127

            for k, i in enumerate(proc_order):
                P = tp(i)
                t0 = i * T

                conf_i = io.tile([128, T, C], F16, tag="conf")
                # split the conf transfer across the three DGE queues
                splits = [0, 43, 86, P]
                dma_engs = [nc.sync, nc.scalar, nc.gpsimd]
                for si in range(3):
                    p0, p1 = splits[si], splits[si + 1]
                    sub = bass.AP(
                        tensor=conf[:, :].tensor,
                        offset=p0 * APP * C + t0 * C,
                        ap=[[APP * C, p1 - p0], [C, T], [1, C]],
                    )
                    dma_engs[si].dma_start(conf_i[p0:p1], sub)
                lt_i = io.tile([128, T, 9], F16, tag="lt")
                nc.gpsimd.dma_start(lt_i[:P], dram_ap(lt, 9, P, t0))

                tlab = lt_i[:P, :, 8:9]  # [P,T,1] f16

                # ---- conf path ----
                e_t = epool.tile([128, T, C], F16, tag="e")
                nc.scalar.activation(e_t[:P], conf_i[:P], AF.Exp)
                s_t = small.tile([128, T], F16, tag="s")
                with nc.allow_low_precision("fp32-internal accum; fp16 write"):
                    nc.vector.reduce_sum(s_t[:P], e_t[:P], axis=AX.X)

                iseq = ipool.tile([128, T, CP], F16, tag="iseq")
                nc.vector.tensor_tensor(
                    iseq[:P],
                    iota16[:P, None, :].broadcast_to([P, T, CP]),
                    tlab.broadcast_to([P, T, CP]),
                    OP.is_equal,
                )
                me = mpool.tile([128, T, C], F16, tag="me")
                nc.vector.tensor_tensor(me[:P], conf_i[:P], iseq[:P, :, 0:C], OP.mult)
                la = small.tile([128, T], F16, tag="la")
                with nc.allow_low_precision("one-hot row sum; fp16 write"):
                    nc.vector.reduce_sum(la[:P], me[:P], axis=AX.X)

                # per-anchor scalars [P, T]
                lns = small.tile([128, T], F32, tag="lns")
                nc.scalar.activation(lns[:P], s_t[:P], AF.Ln)
                nlp = small.tile([128, T], F32, tag="nlp")  # -log pt
                nc.vector.tensor_tensor(nlp[:P], lns[:P], la[:P], OP.subtract)
                pt = small.tile([128, T], F32, tag="pt")
                nc.scalar.activation(pt[:P], nlp[:P], AF.Exp, scale=-1.0)
                usq = small.tile([128, T], F32, tag="usq")  # (1-pt)^2
                nc.scalar.activation(usq[:P], pt[:P], AF.Square, bias=negone[:P])

                wv = small.tile([128, T, 2], F16, tag="wv")
                nc.gpsimd.memset(wv[:P, :, 1:2], 1.0)
                nc.vector.tensor_tensor(wv[:P, :, 0:1], usq[:P, :, None], nlp[:P, :, None], OP.mult)

                for t in range(T):
                    nc.tensor.matmul(
                        ph[:, :],
                        iseq[:P, t, :],
                        wv[:P, t, :],
                        start=(k == 0 and t == 0),
                        stop=(k == n_tiles - 1 and t == T - 1),
                    )

                # ---- loc path ----
                df = small.tile([128, T, 4], F16, tag="df")
                nc.vector.tensor_tensor(df[:P], lt_i[:P, :, 0:4], lt_i[:P, :, 4:8], OP.subtract)
                da = small.tile([128, T, 4], F16, tag="da")
                nc.scalar.activation(da[:P], df[:P], AF.Abs)
                posb = small.tile([128, T], F16, tag="posb")
                nc.vector.tensor_scalar(posb[:P], tlab.squeeze(), 0.0, None, OP.is_gt)
                dp = small.tile([128, T, 4], F16, tag="dp")
                nc.vector.tensor_tensor(
                    dp[:P], da[:P], posb[:P, :, None].broadcast_to([P, T, 4]), OP.mult
                )
                mp = small.tile([128, T, 4], F16, tag="mp")
                nc.vector.tensor_scalar_min(mp[:P], dp[:P], 1.0)
                u_t = small.tile([128, T, 4], F16, tag="u")
                nc.vector.tensor_tensor(u_t[:P], dp[:P], mp[:P], OP.subtract)
                # sum(pos*sl1) = sum(dp - mp) + sum(0.5*mp^2)
                nc.vector.reduce_sum(strip[:P, 2 * k : 2 * k + 1], u_t[:P], axis=AX.XY)
                q_t = small.tile([128, T, 4], F16, tag="q")
                nc.vector.scalar_tensor_tensor(q_t[:P], mp[:P], 0.5, mp[:P], OP.mult, OP.mult)
                nc.vector.reduce_sum(strip[:P, 2 * k + 1 : 2 * k + 2], q_t[:P], axis=AX.XY)

            # ---- finalize ----
            hc = singles.tile([CP, 2], F32)
            nc.vector.tensor_copy(hc[:, :], ph[:, :])
            nc.sync.dma_start(hist[:, :], hc[0:C, :])
            lacc = singles.tile([128, 1], F32)
            nc.vector.reduce_sum(lacc[:, :], strip[:, :], axis=AX.X)
            nc.scalar.dma_start(locs[:, :], lacc[:, :])

    nc.compile()
    return nc


_CACHED = {}


def _get_nc(A, APP, T):
    key = (A, APP, T)
    if key not in _CACHED:
        _CACHED[key] = build_kernel(A, APP, T)
    return _CACHED[key]


def combine_host(hists, locsums, alpha):
    """hists: [ncores, 81, 2]; locsums: [ncores, 128, 1]; alpha: [81]."""
    h = hists[:, :, 0].sum(axis=0).astype(np.float64)
    cnt = hists[:, :, 1].sum(axis=0).astype(np.float64)
    alpha = alpha.astype(np.float64)
    denom = np.clip(alpha * cnt, 1.0, None)
    conf_loss = np.sum(alpha * h / denom)
    num_pos = cnt[1:].sum()
    loc_sum = locsums.astype(np.float64).sum()
    denom_loc = max(num_pos * 4.0, 1.0)
    loc_loss = loc_sum / denom_loc if num_pos > 0 else 0.0
    return np.float32(loc_loss), np.float32(conf_loss)


def kernel(loc_pred, conf_pred, targets, alpha, _trace=False):
    B, A, _ = conf_pred.shape
    assert B == 8 and A == 76725
    nc = _get_nc(A, 600, 75)
    conf16 = np.ascontiguousarray(conf_pred, dtype=np.float16)
    lt16 = np.empty((B, A, 9), dtype=np.float16)
    lt16[:, :, 0:4] = loc_pred
    lt16[:, :, 4:9] = targets
    in_maps = [{"conf": conf16[b], "lt": lt16[b]} for b in range(B)]
    res = run_bass_kernel_spmd(nc, in_maps, core_ids=list(range(B)), trace=_trace)
    hists = np.stack([r["hist"] for r in res.results])
    locsums = np.stack([r["locs"] for r in res.results])
    out = combine_host(hists, locsums, np.asarray(alpha, dtype=np.float32))
    if _trace:
        return out, res
    return out


# revision 12
# speedup vs baseline: 3.1643x; 1.8182x over previous
"""Focal-loss + smooth-L1 loss kernel for TRN2, SPMD over 8 NeuronCores.

Sharding: data-parallel over the batch axis (B=8 -> one batch row per core).
Each core computes, over its 76725 anchors:
  - per-class weighted histogram h[c]   = sum_n [lab_n==c] * (1-pt)^2 * (-logpt)
  - per-class count histogram  cnt[c]   = sum_n [lab_n==c]  (lab=-1 matches nothing)
  - per-partition smooth-L1 partial sums (positives-masked)
The tiny final reduction happens on host.

Inputs are cast to fp16 on host (halves HBM traffic; rel-err impact ~1e-3,
well under the 2e-2 gate). loc/boxes/labels ride one interleaved [A, 9]
tensor so each DMA descriptor is a contiguous 1350B chunk.

Layout: anchor n = 600*p + t (p = SBUF partition, t = free axis), tiles of
T=75 anchors/partition. All input DMAs are issued up-front into resident
SBUF tiles, spread over the three DGE queues (gpsimd SWDGE fans out across
all 16 DMA engines; sync/scalar HWDGE carry one tile each). Tiles are
processed in DMA-arrival order.

Per tile: exp on the scalar engine; row-sum s, logit gather la = conf[lab]
(one-hot multiply + reduce), and smooth-L1 partials on DVE; the one-hot
indicator matrix on gpsimd; per-class histograms via 75 PSUM-accumulated
TensorE matmuls (lhsT = iseq [P,82], rhs = [w, 1] [P,2]).
log pt = la - ln(s); w = (1-pt)^2 * (-log pt).
"""

import numpy as np

import concourse.bass as bass
import concourse.bacc as bacc
import concourse.mybir as mybir
import concourse.tile as tile
from concourse.bass_utils import run_bass_kernel_spmd

F32 = mybir.dt.float32
F16 = mybir.dt.float16
I16 = mybir.dt.int16
AF = mybir.ActivationFunctionType
OP = mybir.AluOpType
AX = mybir.AxisListType

C = 81
CP = 82  # padded classes (even -> aligned fp16 rows; pad col never matches)


def build_kernel(A, APP, T):
    """A anchors, APP anchors/partition, T anchors/partition per tile.
    Tiles 0..n-2 use all 128 partitions; the last tile covers only p<127
    (requires A - 127*APP == (APP//T - 1)*T)."""
    n_tiles = APP // T
    t_full = A - 127 * APP
    assert t_full == (n_tiles - 1) * T, (A, APP, T, t_full)

    nc = bacc.Bacc(None, target_bir_lowering=False)
    conf = nc.dram_tensor("conf", [A, C], F16, kind="ExternalInput")
    lt = nc.dram_tensor("lt", [A, 9], F16, kind="ExternalInput")
    hist = nc.dram_tensor("hist", [C, 2], F32, kind="ExternalOutput")
    locs = nc.dram_tensor("locs", [128, 1], F32, kind="ExternalOutput")

    def dram_ap(h, row_elems, P, t0):
        # anchor n = APP*p + t ; element (n, f) at flat n*row_elems + f
        return bass.AP(
            tensor=h[:, :].tensor,
            offset=t0 * row_elems,
            ap=[[APP * row_elems, P], [row_elems, T], [1, row_elems]],
        )

    # DMA-arrival-aware processing order: gpsimd (SWDGE, fans out over 16 DMA
    # engines) carries six tiles; sync/scalar HWDGE (~25GB/s each) carry one
    # big tile each, which arrives late, so those tiles are processed late.
    gp_tiles = [0, 2, 4, 5, 6, 7]
    proc_order = [0, 2, 4, 5, 6, 1, 3, 7]

    with tile.TileContext(nc) as tc:
        with (
            tc.tile_pool(name="singles", bufs=1) as singles,
            tc.tile_pool(name="epool", bufs=2) as epool,
            tc.tile_pool(name="ipool", bufs=2) as ipool,
            tc.tile_pool(name="mpool", bufs=2) as mpool,
            tc.tile_pool(name="small", bufs=2) as small,
            tc.tile_pool(name="psum", bufs=1, space="PSUM") as psum,
        ):
            # constants
            iota_i = singles.tile([128, CP], I16)
            nc.gpsimd.iota(iota_i[:, :], [[1, CP]], channel_multiplier=0)
            iota16 = singles.tile([128, CP], F16)
            nc.vector.tensor_copy(iota16[:, :], iota_i[:, :])
            strip = singles.tile([128, 2 * n_tiles], F32)
            nc.vector.memset(strip[:, :], 0.0)
            negone = singles.tile([128, 1], F32)
            nc.gpsimd.memset(negone[:, :], -1.0)

            ph = psum.tile([CP, 2], F32)

            # resident input tiles; all DMAs issued up-front
            conf_t = [
                singles.tile([128, T, C], F16, tag=f"conf{i}", name=f"conf_t{i}")
                for i in range(n_tiles)
            ]
            lt_t = [
                singles.tile([128, T, 9], F16, tag=f"lt{i}", name=f"lt_t{i}")
                for i in range(n_tiles)
            ]

            def tp(i):
                return 128 if i < n_tiles - 1 else 127

            for i in proc_order:
                nc.gpsimd.dma_start(lt_t[i][: tp(i)], dram_ap(lt, 9, tp(i), i * T))
            for i in proc_order:
                if i in gp_tiles:
                    nc.gpsimd.dma_start(conf_t[i][: tp(i)], dram_ap(conf, C, tp(i), i * T))
            nc.sync.dma_start(conf_t[1][:128], dram_ap(conf, C, 128, 1 * T))
            nc.scalar.dma_start(conf_t[3][:128], dram_ap(conf, C, 128, 3 * T))

            for k, i in enumerate(proc_order):
                P = tp(i)
                tlab = lt_t[i][:P, :, 8:9]  # [P,T,1] f16

                # ---- conf path ----
                e_t = epool.tile([128, T, C], F16, tag="e")
                nc.scalar.activation(e_t[:P], conf_t[i][:P], AF.Exp)
                s_t = small.tile([128, T], F16, tag="s")
                with nc.allow_low_precision("fp32-internal accum; fp16 write"):
                    nc.vector.reduce_sum(s_t[:P], e_t[:P], axis=AX.X)

                iseq = ipool.tile([128, T, CP], F16, tag="iseq")
                nc.vector.tensor_tensor(
                    iseq[:P],
                    iota16[:P, None, :].broadcast_to([P, T, CP]),
                    tlab.broadcast_to([P, T, CP]),
                    OP.is_equal,
                )
                me = mpool.tile([128, T, C], F16, tag="me")
                nc.vector.tensor_tensor(me[:P], conf_t[i][:P], iseq[:P, :, 0:C], OP.mult)
                la = small.tile([128, T], F16, tag="la")
                with nc.allow_low_precision("one-hot row sum; fp16 write"):
                    nc.vector.reduce_sum(la[:P], me[:P], axis=AX.X)

                # per-anchor scalars [P, T]
                lns = small.tile([128, T], F32, tag="lns")
                nc.scalar.activation(lns[:P], s_t[:P], AF.Ln)
                nlp = small.tile([128, T], F32, tag="nlp")  # -log pt
                nc.vector.tensor_tensor(nlp[:P], lns[:P], la[:P], OP.subtract)
                pt = small.tile([128, T], F32, tag="pt")
                nc.scalar.activation(pt[:P], nlp[:P], AF.Exp, scale=-1.0)
                usq = small.tile([128, T], F32, tag="usq")  # (1-pt)^2
                nc.scalar.activation(usq[:P], pt[:P], AF.Square, bias=negone[:P])

                wv = small.tile([128, T, 2], F16, tag="wv")
                nc.gpsimd.memset(wv[:P, :, 1:2], 1.0)
                nc.vector.tensor_tensor(wv[:P, :, 0:1], usq[:P, :, None], nlp[:P, :, None], OP.mult)

                for t in range(T):
                    nc.tensor.matmul(
                        ph[:, :],
                        iseq[:P, t, :],
                        wv[:P, t, :],
                        start=(k == 0 and t == 0),
                        stop=(k == n_tiles - 1 and t == T - 1),
                    )

                # ---- loc path ----
                df = small.tile([128, T, 4], F16, tag="df")
                nc.vector.tensor_tensor(df[:P], lt_t[i][:P, :, 0:4], lt_t[i][:P, :, 4:8], OP.subtract)
                da = small.tile([128, T, 4], F16, tag="da")
                nc.scalar.activation(da[:P], df[:P], AF.Abs)
                posb = small.tile([128, T], F16, tag="posb")
                nc.vector.tensor_scalar(posb[:P], tlab.squeeze(), 0.0, None, OP.is_gt)
                dp = small.tile([128, T, 4], F16, tag="dp")
                nc.vector.tensor_tensor(
                    dp[:P], da[:P], posb[:P, :, None].broadcast_to([P, T, 4]), OP.mult
                )
                mp = small.tile([128, T, 4], F16, tag="mp")
                nc.vector.tensor_scalar_min(mp[:P], dp[:P], 1.0)
                u_t = small.tile([128, T, 4], F16, tag="u")
                nc.vector.tensor_tensor(u_t[:P], dp[:P], mp[:P], OP.subtract)
                # sum(pos*sl1) = sum(dp - mp) + sum(0.5*mp^2)
                nc.vector.reduce_sum(strip[:P, 2 * k : 2 * k + 1], u_t[:P], axis=AX.XY)
                q_t = small.tile([128, T, 4], F16, tag="q")
                nc.vector.scalar_tensor_tensor(q_t[:P], mp[:P], 0.5, mp[:P], OP.mult, OP.mult)
                nc.vector.reduce_sum(strip[:P, 2 * k + 1 : 2 * k + 2], q_t[:P], axis=AX.XY)

            # ---- finalize ----
            hc = singles.tile([CP, 2], F32)
            nc.vector.tensor_copy(hc[:, :], ph[:, :])
            nc.sync.dma_start(hist[:, :], hc[0:C, :])
            lacc = singles.tile([128, 1], F32)
            nc.vector.reduce_sum(lacc[:, :], strip[:, :], axis=AX.X)
            nc.scalar.dma_start(locs[:, :], lacc[:, :])

    nc.compile()
    return nc


_CACHED = {}


def _get_nc(A, APP, T):
    key = (A, APP, T)
    if key not in _CACHED:
        _CACHED[key] = build_kernel(A, APP, T)
    return _CACHED[key]


def combine_host(hists, locsums, alpha):
    """hists: [ncores, 81, 2]; locsums: [ncores, 128, 1]; alpha: [81]."""
    h = hists[:, :, 0].sum(axis=0).astype(np.float64)
    cnt = hists[:, :, 1].sum(axis=0).astype(np.float64)
    alpha = alpha.astype(np.float64)
    denom = np.clip(alpha * cnt, 1.0, None)
    conf_loss = np.sum(alpha * h / denom)
    num_pos = cnt[1:].sum()
    loc_sum = locsums.astype(np.float64).sum()
    denom_loc = max(num_pos * 4.0, 1.0)
    loc_loss = loc_sum / denom_loc if num_pos > 0 else 0.0
    return np.float32(loc_loss), np.float32(conf_loss)


def kernel(loc_pred, conf_pred, targets, alpha, _trace=False):
    B, A, _ = conf_pred.shape
    assert B == 8 and A == 76725
    nc = _get_nc(A, 600, 75)
    conf16 = np.ascontiguousarray(conf_pred, dtype=np.float16)
    lt16 = np.empty((B, A, 9), dtype=np.float16)
    lt16[:, :, 0:4] = loc_pred
    lt16[:, :, 4:9] = targets
    in_maps = [{"conf": conf16[b], "lt": lt16[b]} for b in range(B)]
    res = run_bass_kernel_spmd(nc, in_maps, core_ids=list(range(B)), trace=_trace)
    hists = np.stack([r["hist"] for r in res.results])
    locsums = np.stack([r["locs"] for r in res.results])
    out = combine_host(hists, locsums, np.asarray(alpha, dtype=np.float32))
    if _trace:
        return out, res
    return out


# revision 23
# speedup vs baseline: 3.4156x; 1.0794x over previous
"""Focal-loss + smooth-L1 loss kernel for TRN2, SPMD over 8 NeuronCores.

Sharding: data-parallel over the batch axis (B=8 -> one batch row per core).
Each core computes, over its 76725 anchors:
  - per-class weighted histogram h[c]   = sum_n [lab_n==c] * (1-pt)^2 * (-logpt)
  - per-class count histogram  cnt[c]   = sum_n [lab_n==c]  (lab=-1 matches nothing)
  - per-partition smooth-L1 partial sums (positives-masked)
The tiny final reduction happens on host.

Inputs are cast to fp16 on host (halves HBM traffic; rel-err ~1e-3, well
under the 2e-2 gate). loc/boxes/labels ride one interleaved [A, 9] tensor.

Layout: anchor n = 600*p + t (p = SBUF partition, t = free axis), tiles of
T=75 anchors/partition. All input DMAs are issued up-front into resident
SBUF tiles on the gpsimd SWDGE queue (fans out across all 16 DMA engines,
~120GB/s measured; the HWDGE queues are descriptor-rate-limited to
~1desc/us and are only used for the tiny outputs).

Per tile: exp on the scalar engine. Row sums (s = sum_c e, la = conf[lab])
use in-place fold trees: TensorTensor adds run at 2x on packed fp16 while
TensorReduce measures 1x, so fold 81 -> 40 -> 20 -> 10 then reduce.
log pt = la - ln(s); w = (1-pt)^2 * (-log pt).

Histograms: PSUM-accumulated TensorE matmuls grouped Q=6 anchors per
instruction: lhsT = wv [P, 6*2], rhs = iseq [P, 6*82], out [12, 492].
Off-diagonal (tq != tq') blocks are garbage that is simply never read;
the 6 diagonal [2, 82] blocks are summed once at the end. This cuts the
per-anchor PE cost from ~397ns (600 tiny matmuls, weight-load bound) to
~60ns (104 grouped matmuls).
"""

import numpy as np

import concourse.bass as bass
import concourse.bacc as bacc
import concourse.mybir as mybir
import concourse.tile as tile
from concourse.bass_utils import run_bass_kernel_spmd

F32 = mybir.dt.float32
F16 = mybir.dt.float16
I16 = mybir.dt.int16
AF = mybir.ActivationFunctionType
OP = mybir.AluOpType
AX = mybir.AxisListType

C = 81
CP = 82  # padded classes (pad col never matches any label)
Q = 6  # anchors (t-steps) per histogram matmul


def fold_sum(nc, x, out, P, width):
    """out[P,T] = sum over last axis of x[P,T,width] via in-place fold tree
    (TensorTensor adds hit 2x on packed fp16; TensorReduce only runs 1x).
    width=81: fold 40+40, 20+20, 10+10, reduce 10, add col 80."""
    assert width == 81
    nc.vector.tensor_tensor(x[:P, :, 0:40], x[:P, :, 0:40], x[:P, :, 40:80], OP.add)
    nc.vector.tensor_tensor(x[:P, :, 0:20], x[:P, :, 0:20], x[:P, :, 20:40], OP.add)
    nc.vector.tensor_tensor(x[:P, :, 0:10], x[:P, :, 0:10], x[:P, :, 10:20], OP.add)
    nc.vector.reduce_sum(out[:P], x[:P, :, 0:10], axis=AX.X)
    nc.vector.tensor_tensor(out[:P], out[:P], x[:P, :, 80:81].squeeze(), OP.add)


def build_kernel(A, APP, T):
    """A anchors, APP anchors/partition, T anchors/partition per tile.
    Tiles 0..n-2 use all 128 partitions; the last tile covers only p<127
    (requires A - 127*APP == (APP//T - 1)*T)."""
    n_tiles = APP // T
    t_full = A - 127 * APP
    assert t_full == (n_tiles - 1) * T, (A, APP, T, t_full)
    n_full_g, tail = divmod(T, Q)  # 12 groups of 6, tail 3

    nc = bacc.Bacc(None, target_bir_lowering=False)
    conf = nc.dram_tensor("conf", [A, C], F16, kind="ExternalInput")
    lt = nc.dram_tensor("lt", [A, 9], F16, kind="ExternalInput")
    tail_g = T % Q
    hist6 = nc.dram_tensor("hist6", [2 * Q, CP * Q], F32, kind="ExternalOutput")
    hist3 = nc.dram_tensor("hist3", [2 * tail_g, CP * tail_g], F32, kind="ExternalOutput")
    locs = nc.dram_tensor("locs", [128, 1], F32, kind="ExternalOutput")

    def dram_ap(h, row_elems, P, t0):
        # anchor n = APP*p + t ; element (n, f) at flat n*row_elems + f
        return bass.AP(
            tensor=h[:, :].tensor,
            offset=t0 * row_elems,
            ap=[[APP * row_elems, P], [row_elems, T], [1, row_elems]],
        )

    with tile.TileContext(nc) as tc:
        with (
            tc.tile_pool(name="singles", bufs=1) as singles,
            tc.tile_pool(name="epool", bufs=2) as epool,
            tc.tile_pool(name="ipool", bufs=2) as ipool,
            tc.tile_pool(name="mpool", bufs=2) as mpool,
            tc.tile_pool(name="small", bufs=2) as small,
            tc.tile_pool(name="psum", bufs=1, space="PSUM") as psum,
        ):
            # constants
            iota_i = singles.tile([128, CP], I16)
            nc.gpsimd.iota(iota_i[:, :], [[1, CP]], channel_multiplier=0)
            iota16 = singles.tile([128, CP], F16)
            nc.vector.tensor_copy(iota16[:, :], iota_i[:, :])
            strip = singles.tile([128, 2 * n_tiles], F32)
            nc.vector.memset(strip[:, :], 0.0)
            negone = singles.tile([128, 1], F32)
            nc.gpsimd.memset(negone[:, :], -1.0)

            ph6 = psum.tile([2 * Q, CP * Q], F32)
            ph3 = (
                psum.tile([2 * (T % Q), CP * (T % Q)], F32, name="ph3")
                if T % Q
                else None
            )

            # resident input tiles; all DMAs issued up-front on the SWDGE queue
            conf_t = [
                singles.tile([128, T, C], F16, tag=f"conf{i}", name=f"conf_t{i}")
                for i in range(n_tiles)
            ]
            lt_t = [
                singles.tile([128, T, 9], F16, tag=f"lt{i}", name=f"lt_t{i}")
                for i in range(n_tiles)
            ]

            def tp(i):
                return 128 if i < n_tiles - 1 else 127

            for i in range(n_tiles):
                nc.gpsimd.dma_start(lt_t[i][: tp(i)], dram_ap(lt, 9, tp(i), i * T))
                nc.gpsimd.dma_start(conf_t[i][: tp(i)], dram_ap(conf, C, tp(i), i * T))

            for k in range(n_tiles):
                i = k
                P = tp(i)
                tlab = lt_t[i][:P, :, 8:9]  # [P,T,1] f16

                # ---- conf path ----
                e_t = epool.tile([128, T, C], F16, tag="e")
                nc.scalar.activation(e_t[:P], conf_t[i][:P], AF.Exp)
                s_t = small.tile([128, T], F16, tag="s")
                with nc.allow_low_precision("fp32-internal accum; fp16 write"):
                    fold_sum(nc, e_t, s_t, P, C)

                iseq = ipool.tile([128, T, CP], F16, tag="iseq")
                nc.vector.tensor_tensor(
                    iseq[:P],
                    iota16[:P, None, :].broadcast_to([P, T, CP]),
                    tlab.broadcast_to([P, T, CP]),
                    OP.is_equal,
                )
                me = mpool.tile([128, T, C], F16, tag="me")
                nc.vector.tensor_tensor(me[:P], conf_t[i][:P], iseq[:P, :, 0:C], OP.mult)
                la = small.tile([128, T], F16, tag="la")
                with nc.allow_low_precision("one-hot row sum; fp16 write"):
                    fold_sum(nc, me, la, P, C)

                # per-anchor scalars [P, T]
                lns = small.tile([128, T], F32, tag="lns")
                nc.scalar.activation(lns[:P], s_t[:P], AF.Ln)
                nlp = small.tile([128, T], F32, tag="nlp")  # -log pt
                nc.vector.tensor_tensor(nlp[:P], lns[:P], la[:P], OP.subtract)
                pt = small.tile([128, T], F32, tag="pt")
                nc.scalar.activation(pt[:P], nlp[:P], AF.Exp, scale=-1.0)
                usq = small.tile([128, T], F32, tag="usq")  # (1-pt)^2
                nc.scalar.activation(usq[:P], pt[:P], AF.Square, bias=negone[:P])

                wv = small.tile([128, T, 2], F16, tag="wv")
                nc.gpsimd.memset(wv[:P, :, 1:2], 1.0)
                nc.vector.tensor_tensor(wv[:P, :, 0:1], usq[:P, :, None], nlp[:P, :, None], OP.mult)

                # ---- histogram matmuls, Q anchors per instruction ----
                for g in range(n_full_g):
                    t0 = g * Q
                    nc.tensor.matmul(
                        ph6[:, :],
                        wv[:P, t0 : t0 + Q, :],
                        iseq[:P, t0 : t0 + Q, :],
                        start=(k == 0 and g == 0),
                        stop=(k == n_tiles - 1 and g == n_full_g - 1),
                    )
                if tail:
                    t0 = n_full_g * Q
                    nc.tensor.matmul(
                        ph3[:, :],
                        wv[:P, t0 : t0 + tail, :],
                        iseq[:P, t0 : t0 + tail, :],
                        start=(k == 0),
                        stop=(k == n_tiles - 1),
                    )

                # ---- loc path ----
                df = small.tile([128, T, 4], F16, tag="df")
                nc.vector.tensor_tensor(df[:P], lt_t[i][:P, :, 0:4], lt_t[i][:P, :, 4:8], OP.subtract)
                da = small.tile([128, T, 4], F16, tag="da")
                nc.scalar.activation(da[:P], df[:P], AF.Abs)
                posb = small.tile([128, T], F16, tag="posb")
                nc.vector.tensor_scalar(posb[:P], tlab.squeeze(), 0.0, None, OP.is_gt)
                dp = small.tile([128, T, 4], F16, tag="dp")
                nc.vector.tensor_tensor(
                    dp[:P], da[:P], posb[:P, :, None].broadcast_to([P, T, 4]), OP.mult
                )
                mp = small.tile([128, T, 4], F16, tag="mp")
                nc.vector.tensor_scalar_min(mp[:P], dp[:P], 1.0)
                u_t = small.tile([128, T, 4], F16, tag="u")
                nc.vector.tensor_tensor(u_t[:P], dp[:P], mp[:P], OP.subtract)
                # sum(pos*sl1) = sum(dp - mp) + sum(0.5*mp^2)
                nc.vector.reduce_sum(strip[:P, 2 * k : 2 * k + 1], u_t[:P], axis=AX.XY)
                q_t = small.tile([128, T, 4], F16, tag="q")
                nc.vector.scalar_tensor_tensor(q_t[:P], mp[:P], 0.5, mp[:P], OP.mult, OP.mult)
                nc.vector.reduce_sum(strip[:P, 2 * k + 1 : 2 * k + 2], q_t[:P], axis=AX.XY)

            # ---- finalize: ship raw PSUM blocks; host extracts diagonals ----
            hps = singles.tile([2 * Q, CP * Q], F32)
            nc.vector.tensor_copy(hps[:, :], ph6[:, :])
            nc.sync.dma_start(hist6[:, :], hps[:, :])
            if tail:
                hps3 = singles.tile([2 * tail, CP * tail], F32)
                nc.vector.tensor_copy(hps3[:, :], ph3[:, :])
                nc.sync.dma_start(hist3[:, :], hps3[:, :])
            lacc = singles.tile([128, 1], F32)
            nc.vector.reduce_sum(lacc[:, :], strip[:, :], axis=AX.X)
            nc.scalar.dma_start(locs[:, :], lacc[:, :])

    nc.compile()
    return nc


_CACHED = {}


def _get_nc(A, APP, T):
    key = (A, APP, T)
    if key not in _CACHED:
        _CACHED[key] = build_kernel(A, APP, T)
    return _CACHED[key]


def extract_diag(blk, q):
    """blk: [ncores, 2q, 82q] grouped-matmul PSUM dump -> [ncores, 2, 81]
    by summing the q diagonal [2, 82] blocks (off-diagonals are garbage)."""
    nc_, _, _ = blk.shape
    out = np.zeros((nc_, 2, C), dtype=np.float64)
    for tq in range(q):
        out += blk[:, 2 * tq : 2 * tq + 2, CP * tq : CP * tq + C]
    return out


def combine_host(hists, locsums, alpha):
    """hists: [ncores, 2, 81] (row 0 = weighted, row 1 = counts);
    locsums: [ncores, 128, 1]; alpha: [81]."""
    h = hists[:, 0, :].sum(axis=0).astype(np.float64)
    cnt = hists[:, 1, :].sum(axis=0).astype(np.float64)
    alpha = alpha.astype(np.float64)
    denom = np.clip(alpha * cnt, 1.0, None)
    conf_loss = np.sum(alpha * h / denom)
    num_pos = cnt[1:].sum()
    loc_sum = locsums.astype(np.float64).sum()
    denom_loc = max(num_pos * 4.0, 1.0)
    loc_loss = loc_sum / denom_loc if num_pos > 0 else 0.0
    return np.float32(loc_loss), np.float32(conf_loss)


def kernel(loc_pred, conf_pred, targets, alpha, _trace=False):
    B, A, _ = conf_pred.shape
    assert B == 8 and A == 76725
    nc = _get_nc(A, 600, 75)
    conf16 = np.ascontiguousarray(conf_pred, dtype=np.float16)
    lt16 = np.empty((B, A, 9), dtype=np.float16)
    lt16[:, :, 0:4] = loc_pred
    lt16[:, :, 4:9] = targets
    in_maps = [{"conf": conf16[b], "lt": lt16[b]} for b in range(B)]
    res = run_bass_kernel_spmd(nc, in_maps, core_ids=list(range(B)), trace=_trace)
    h6 = np.stack([r["hist6"] for r in res.results]).astype(np.float64)
    h3 = np.stack([r["hist3"] for r in res.results]).astype(np.float64)
    hists = extract_diag(h6, 6) + extract_diag(h3, 3)
    locsums = np.stack([r["locs"] for r in res.results])
    out = combine_host(hists, locsums, np.asarray(alpha, dtype=np.float32))
    if _trace:
        return out, res
    return out


# revision 24
# speedup vs baseline: 3.9462x; 1.1553x over previous
"""Focal-loss + smooth-L1 loss kernel for TRN2, SPMD over 8 NeuronCores.

Sharding: data-parallel over the batch axis (B=8 -> one batch row per core).
Each core computes, over its 76725 anchors:
  - per-class weighted histogram h[c]   = sum_n [lab_n==c] * (1-pt)^2 * (-logpt)
  - per-class count histogram  cnt[c]   = sum_n [lab_n==c]  (lab=-1 matches nothing)
  - total positives-masked smooth-L1 sum
The tiny final reduction happens on host.

Inputs are cast to fp16 on host (halves HBM traffic; rel-err ~1e-3, well
under the 2e-2 gate). loc/boxes/labels ride one interleaved [A, 9] tensor.

Layout: anchor n = 600*p + t (p = SBUF partition, t = free axis), tiles of
T=75 anchors/partition. All input DMAs are issued up-front into resident
SBUF tiles on the gpsimd SWDGE queue (fans out across all 16 DMA engines,
~150GB/s measured; HWDGE queues are descriptor-rate-limited to ~1desc/us
and only carry the tiny outputs).

Per tile: exp on the scalar engine. Row sums (s = sum_c e, la = conf[lab])
use in-place fold trees with even-aligned fp16 slices (TensorTensor adds
run at 2x on packed fp16 when slice offsets are even; TensorReduce only
runs 1x): fold 82 -> 40 -> 20 -> 10 -> reduce.
log pt = la - ln(s); w = (1-pt)^2 * (-log pt).

All three loss sums ride ONE grouped TensorE matmul per Q=6 anchors:
lhsT = wv [P, Q*3] with per-anchor columns [w, 1, sl1_sum], rhs = iseq
[P, Q*82], PSUM out [18, 492] accumulated over all tiles. Off-diagonal
(tq != tq') blocks are garbage that is never read; the host sums the Q
diagonal [3, 82] blocks: row 0 -> h[c], row 1 -> cnt[c], row 2 summed
over c>=1 -> the positives-masked smooth-L1 total (the one-hot columns
provide the label mask for free). This cuts the per-anchor PE cost from
~397ns (600 tiny matmuls, weight-load bound) to ~60ns and removes the
positive-mask / strip-reduction DVE ops entirely.
"""

import numpy as np

import concourse.bass as bass
import concourse.bacc as bacc
import concourse.mybir as mybir
import concourse.tile as tile
from concourse.bass_utils import run_bass_kernel_spmd

F32 = mybir.dt.float32
F16 = mybir.dt.float16
I16 = mybir.dt.int16
AF = mybir.ActivationFunctionType
OP = mybir.AluOpType
AX = mybir.AxisListType

C = 81
CP = 82  # padded classes (pad col never matches any label)
Q = 6  # anchors (t-steps) per histogram matmul


def fold_sum(nc, x, out, P):
    """out[P,T] = sum over last axis of x[P,T,82] (col 81 must be zero) via
    in-place fold tree. All slice offsets/counts even so packed-fp16
    TensorTensor adds hit 2x; the 1x TensorReduce only sees width 10."""
    nc.vector.tensor_tensor(x[:P, :, 0:40], x[:P, :, 0:40], x[:P, :, 42:82], OP.add)
    nc.vector.tensor_tensor(x[:P, :, 0:2], x[:P, :, 0:2], x[:P, :, 40:42], OP.add)
    nc.vector.tensor_tensor(x[:P, :, 0:20], x[:P, :, 0:20], x[:P, :, 20:40], OP.add)
    nc.vector.tensor_tensor(x[:P, :, 0:10], x[:P, :, 0:10], x[:P, :, 10:20], OP.add)
    nc.vector.reduce_sum(out[:P], x[:P, :, 0:10], axis=AX.X)


def build_kernel(A, APP, T):
    """A anchors, APP anchors/partition, T anchors/partition per tile.
    Tiles 0..n-2 use all 128 partitions; the last tile covers only p<127
    (requires A - 127*APP == (APP//T - 1)*T)."""
    n_tiles = APP // T
    t_full = A - 127 * APP
    assert t_full == (n_tiles - 1) * T, (A, APP, T, t_full)
    n_full_g, tail = divmod(T, Q)  # 12 groups of 6, tail 3

    nc = bacc.Bacc(None, target_bir_lowering=False)
    conf = nc.dram_tensor("conf", [A, C], F16, kind="ExternalInput")
    lt = nc.dram_tensor("lt", [A, 9], F16, kind="ExternalInput")
    hist6 = nc.dram_tensor("hist6", [3 * Q, CP * Q], F32, kind="ExternalOutput")
    hist3 = nc.dram_tensor("hist3", [3 * tail, CP * tail], F32, kind="ExternalOutput")

    def dram_ap(h, row_elems, P, t0):
        # anchor n = APP*p + t ; element (n, f) at flat n*row_elems + f
        return bass.AP(
            tensor=h[:, :].tensor,
            offset=t0 * row_elems,
            ap=[[APP * row_elems, P], [row_elems, T], [1, row_elems]],
        )

    with tile.TileContext(nc) as tc:
        with (
            tc.tile_pool(name="singles", bufs=1) as singles,
            tc.tile_pool(name="epool", bufs=2) as epool,
            tc.tile_pool(name="ipool", bufs=2) as ipool,
            tc.tile_pool(name="mpool", bufs=2) as mpool,
            tc.tile_pool(name="small", bufs=2) as small,
            tc.tile_pool(name="psum", bufs=1, space="PSUM") as psum,
        ):
            # constants
            iota_i = singles.tile([128, CP], I16)
            nc.gpsimd.iota(iota_i[:, :], [[1, CP]], channel_multiplier=0)
            iota16 = singles.tile([128, CP], F16)
            nc.vector.tensor_copy(iota16[:, :], iota_i[:, :])
            negone = singles.tile([128, 1], F32)
            nc.gpsimd.memset(negone[:, :], -1.0)

            ph6 = psum.tile([3 * Q, CP * Q], F32)
            ph3 = psum.tile([3 * tail, CP * tail], F32, name="ph3")

            # resident input tiles; all DMAs issued up-front on the SWDGE queue
            conf_t = [
                singles.tile([128, T, C], F16, tag=f"conf{i}", name=f"conf_t{i}")
                for i in range(n_tiles)
            ]
            lt_t = [
                singles.tile([128, T, 9], F16, tag=f"lt{i}", name=f"lt_t{i}")
                for i in range(n_tiles)
            ]

            def tp(i):
                return 128 if i < n_tiles - 1 else 127

            for i in range(n_tiles):
                nc.gpsimd.dma_start(lt_t[i][: tp(i)], dram_ap(lt, 9, tp(i), i * T))
                nc.gpsimd.dma_start(conf_t[i][: tp(i)], dram_ap(conf, C, tp(i), i * T))

            for k in range(n_tiles):
                i = k
                P = tp(i)
                tlab = lt_t[i][:P, :, 8:9]  # [P,T,1] f16

                # ---- conf path ----
                e_t = epool.tile([128, T, CP], F16, tag="e")
                nc.gpsimd.memset(e_t[:P, :, 81:82], 0.0)
                nc.scalar.activation(e_t[:P, :, 0:C], conf_t[i][:P], AF.Exp)
                s_t = small.tile([128, T], F16, tag="s")
                with nc.allow_low_precision("fp32-internal accum; fp16 write"):
                    fold_sum(nc, e_t, s_t, P)

                iseq = ipool.tile([128, T, CP], F16, tag="iseq")
                nc.vector.tensor_tensor(
                    iseq[:P],
                    iota16[:P, None, :].broadcast_to([P, T, CP]),
                    tlab.broadcast_to([P, T, CP]),
                    OP.is_equal,
                )
                me = mpool.tile([128, T, CP], F16, tag="me")
                nc.gpsimd.memset(me[:P, :, 81:82], 0.0)
                nc.vector.tensor_tensor(
                    me[:P, :, 0:C], conf_t[i][:P], iseq[:P, :, 0:C], OP.mult
                )
                la = small.tile([128, T], F16, tag="la")
                with nc.allow_low_precision("one-hot row sum; fp16 write"):
                    fold_sum(nc, me, la, P)

                # per-anchor scalars [P, T]
                lns = small.tile([128, T], F32, tag="lns")
                nc.scalar.activation(lns[:P], s_t[:P], AF.Ln)
                nlp = small.tile([128, T], F32, tag="nlp")  # -log pt
                nc.vector.tensor_tensor(nlp[:P], lns[:P], la[:P], OP.subtract)
                pt = small.tile([128, T], F32, tag="pt")
                nc.scalar.activation(pt[:P], nlp[:P], AF.Exp, scale=-1.0)
                usq = small.tile([128, T], F32, tag="usq")  # (1-pt)^2
                nc.scalar.activation(usq[:P], pt[:P], AF.Square, bias=negone[:P])

                # ---- smooth-L1 per-anchor sum (mask applied by the one-hot) ----
                df = small.tile([128, T, 4], F16, tag="df")
                nc.vector.tensor_tensor(df[:P], lt_t[i][:P, :, 0:4], lt_t[i][:P, :, 4:8], OP.subtract)
                da = small.tile([128, T, 4], F16, tag="da")
                nc.scalar.activation(da[:P], df[:P], AF.Abs)
                mp = small.tile([128, T, 4], F16, tag="mp")
                nc.vector.tensor_scalar_min(mp[:P], da[:P], 1.0)
                u_t = small.tile([128, T, 4], F16, tag="u")
                nc.vector.tensor_tensor(u_t[:P], da[:P], mp[:P], OP.subtract)
                # sl1 = (da - mp) + 0.5*mp^2
                sl = small.tile([128, T, 4], F16, tag="sl")
                nc.vector.scalar_tensor_tensor(sl[:P], mp[:P], 0.5, mp[:P], OP.mult, OP.mult)
                nc.vector.tensor_tensor(sl[:P], sl[:P], u_t[:P], OP.add)

                wv = small.tile([128, T, 3], F16, tag="wv")
                nc.gpsimd.memset(wv[:P, :, 1:2], 1.0)
                nc.vector.tensor_tensor(wv[:P, :, 0:1], usq[:P, :, None], nlp[:P, :, None], OP.mult)
                with nc.allow_low_precision("4-elem row sum; fp16 write"):
                    nc.vector.reduce_sum(wv[:P, :, 2:3], sl[:P], axis=AX.X)

                # ---- the grouped matmuls: h, cnt, loc-sum in one PSUM accum ----
                for g in range(n_full_g):
                    t0 = g * Q
                    nc.tensor.matmul(
                        ph6[:, :],
                        wv[:P, t0 : t0 + Q, :],
                        iseq[:P, t0 : t0 + Q, :],
                        start=(k == 0 and g == 0),
                        stop=(k == n_tiles - 1 and g == n_full_g - 1),
                    )
                if tail:
                    t0 = n_full_g * Q
                    nc.tensor.matmul(
                        ph3[:, :],
                        wv[:P, t0 : t0 + tail, :],
                        iseq[:P, t0 : t0 + tail, :],
                        start=(k == 0),
                        stop=(k == n_tiles - 1),
                    )

            # ---- finalize: ship raw PSUM blocks; host extracts diagonals ----
            hps = singles.tile([3 * Q, CP * Q], F32)
            nc.vector.tensor_copy(hps[:, :], ph6[:, :])
            nc.sync.dma_start(hist6[:, :], hps[:, :])
            if tail:
                hps3 = singles.tile([3 * tail, CP * tail], F32)
                nc.vector.tensor_copy(hps3[:, :], ph3[:, :])
                nc.sync.dma_start(hist3[:, :], hps3[:, :])

    nc.compile()
    return nc


_CACHED = {}


def _get_nc(A, APP, T):
    key = (A, APP, T)
    if key not in _CACHED:
        _CACHED[key] = build_kernel(A, APP, T)
    return _CACHED[key]


def extract_diag(blk, q):
    """blk: [ncores, 3q, 82q] grouped-matmul PSUM dump -> [ncores, 3, 81]
    by summing the q diagonal [3, 82] blocks (off-diagonals are garbage)."""
    nc_, _, _ = blk.shape
    out = np.zeros((nc_, 3, C), dtype=np.float64)
    for tq in range(q):
        out += blk[:, 3 * tq : 3 * tq + 3, CP * tq : CP * tq + C]
    return out


def combine_host(hists, alpha):
    """hists: [ncores, 3, 81] (rows: weighted, counts, sl1-sums); alpha: [81]."""
    h = hists[:, 0, :].sum(axis=0)
    cnt = hists[:, 1, :].sum(axis=0)
    alpha = alpha.astype(np.float64)
    denom = np.clip(alpha * cnt, 1.0, None)
    conf_loss = np.sum(alpha * h / denom)
    num_pos = cnt[1:].sum()
    loc_sum = hists[:, 2, 1:].sum()  # c>=1 selects positive anchors
    denom_loc = max(num_pos * 4.0, 1.0)
    loc_loss = loc_sum / denom_loc if num_pos > 0 else 0.0
    return np.float32(loc_loss), np.float32(conf_loss)


def kernel(loc_pred, conf_pred, targets, alpha, _trace=False):
    B, A, _ = conf_pred.shape
    assert B == 8 and A == 76725
    nc = _get_nc(A, 600, 75)
    conf16 = np.ascontiguousarray(conf_pred, dtype=np.float16)
    lt16 = np.empty((B, A, 9), dtype=np.float16)
    lt16[:, :, 0:4] = loc_pred
    lt16[:, :, 4:9] = targets
    in_maps = [{"conf": conf16[b], "lt": lt16[b]} for b in range(B)]
    res = run_bass_kernel_spmd(nc, in_maps, core_ids=list(range(B)), trace=_trace)
    h6 = np.stack([r["hist6"] for r in res.results]).astype(np.float64)
    h3 = np.stack([r["hist3"] for r in res.results]).astype(np.float64)
    hists = extract_diag(h6, 6) + extract_diag(h3, 3)
    out = combine_host(hists, np.asarray(alpha, dtype=np.float32))
    if _trace:
        return out, res
    return out


# revision 25
# speedup vs baseline: 3.9691x; 1.0058x over previous
"""Focal-loss + smooth-L1 loss kernel for TRN2, SPMD over 8 NeuronCores.

Sharding: data-parallel over the batch axis (B=8 -> one batch row per core).
Each core computes, over its 76725 anchors:
  - per-class weighted histogram h[c]   = sum_n [lab_n==c] * (1-pt)^2 * (-logpt)
  - per-class count histogram  cnt[c]   = sum_n [lab_n==c]  (lab=-1 matches nothing)
  - total positives-masked smooth-L1 sum
The tiny final reduction happens on host.

Inputs are cast to fp16 on host (halves HBM traffic; rel-err ~1e-3, well
under the 2e-2 gate). loc/boxes/labels ride one interleaved [A, 9] tensor.

Layout: anchor n = 600*p + t (p = SBUF partition, t = free axis), tiles of
T=75 anchors/partition. All input DMAs are issued up-front into resident
SBUF tiles on the gpsimd SWDGE queue (fans out across all 16 DMA engines,
~150GB/s measured; HWDGE queues are descriptor-rate-limited to ~1desc/us
and only carry the tiny outputs).

Per tile: exp on the scalar engine. Row sums (s = sum_c e, la = conf[lab])
use in-place fold trees with even-aligned fp16 slices (TensorTensor adds
run at 2x on packed fp16 when slice offsets are even; TensorReduce only
runs 1x): fold 82 -> 40 -> 20 -> 10 -> reduce.
log pt = la - ln(s); w = (1-pt)^2 * (-log pt).

All three loss sums ride ONE grouped TensorE matmul per Q=6 anchors:
lhsT = wv [P, Q*3] with per-anchor columns [w, 1, sl1_sum], rhs = iseq
[P, Q*82], PSUM out [18, 492] accumulated over all tiles. Off-diagonal
(tq != tq') blocks are garbage that is never read; the host sums the Q
diagonal [3, 82] blocks: row 0 -> h[c], row 1 -> cnt[c], row 2 summed
over c>=1 -> the positives-masked smooth-L1 total (the one-hot columns
provide the label mask for free). This cuts the per-anchor PE cost from
~397ns (600 tiny matmuls, weight-load bound) to ~60ns and removes the
positive-mask / strip-reduction DVE ops entirely.
"""

import numpy as np

import concourse.bass as bass
import concourse.bacc as bacc
import concourse.mybir as mybir
import concourse.tile as tile
from concourse.bass_utils import run_bass_kernel_spmd

F32 = mybir.dt.float32
F16 = mybir.dt.float16
I16 = mybir.dt.int16
AF = mybir.ActivationFunctionType
OP = mybir.AluOpType
AX = mybir.AxisListType

C = 81
CP = 82  # padded classes (pad col never matches any label)
Q = 6  # anchors (t-steps) per histogram matmul


def fold_sum(nc, x, out, P):
    """out[P,T] = sum over last axis of x[P,T,82] (col 81 must be zero) via
    in-place fold tree. All slice offsets/counts even so packed-fp16
    TensorTensor adds hit 2x; the 1x TensorReduce only sees width 10."""
    nc.vector.tensor_tensor(x[:P, :, 0:40], x[:P, :, 0:40], x[:P, :, 42:82], OP.add)
    nc.vector.tensor_tensor(x[:P, :, 0:2], x[:P, :, 0:2], x[:P, :, 40:42], OP.add)
    nc.vector.tensor_tensor(x[:P, :, 0:20], x[:P, :, 0:20], x[:P, :, 20:40], OP.add)
    nc.vector.tensor_tensor(x[:P, :, 0:10], x[:P, :, 0:10], x[:P, :, 10:20], OP.add)
    nc.vector.reduce_sum(out[:P], x[:P, :, 0:10], axis=AX.X)


def build_kernel(A, APP, T):
    """A anchors, APP anchors/partition, T anchors/partition per tile.
    Tiles 0..n-2 use all 128 partitions; the last tile covers only p<127
    (requires A - 127*APP == (APP//T - 1)*T)."""
    n_tiles = APP // T
    t_full = A - 127 * APP
    assert t_full == (n_tiles - 1) * T, (A, APP, T, t_full)
    n_full_g, tail = divmod(T, Q)  # 12 groups of 6, tail 3

    nc = bacc.Bacc(None, target_bir_lowering=False)
    conf = nc.dram_tensor("conf", [A, C], F16, kind="ExternalInput")
    lt = nc.dram_tensor("lt", [A, 9], F16, kind="ExternalInput")
    hist6 = nc.dram_tensor("hist6", [3 * Q, CP * Q], F32, kind="ExternalOutput")
    hist3 = nc.dram_tensor("hist3", [3 * tail, CP * tail], F32, kind="ExternalOutput")

    def dram_ap(h, row_elems, P, t0):
        # anchor n = APP*p + t ; element (n, f) at flat n*row_elems + f
        return bass.AP(
            tensor=h[:, :].tensor,
            offset=t0 * row_elems,
            ap=[[APP * row_elems, P], [row_elems, T], [1, row_elems]],
        )

    with tile.TileContext(nc) as tc:
        with (
            tc.tile_pool(name="singles", bufs=1) as singles,
            tc.tile_pool(name="epool", bufs=2) as epool,
            tc.tile_pool(name="ipool", bufs=2) as ipool,
            tc.tile_pool(name="mpool", bufs=2) as mpool,
            tc.tile_pool(name="small", bufs=2) as small,
            tc.tile_pool(name="psum", bufs=1, space="PSUM") as psum,
        ):
            # constants
            iota_i = singles.tile([128, CP], I16)
            nc.gpsimd.iota(iota_i[:, :], [[1, CP]], channel_multiplier=0)
            iota16 = singles.tile([128, CP], F16)
            nc.vector.tensor_copy(iota16[:, :], iota_i[:, :])
            negone = singles.tile([128, 1], F32)
            nc.gpsimd.memset(negone[:, :], -1.0)

            ph6 = psum.tile([3 * Q, CP * Q], F32)
            ph3 = psum.tile([3 * tail, CP * tail], F32, name="ph3")

            # resident input tiles; all DMAs issued up-front on the SWDGE queue
            conf_t = [
                singles.tile([128, T, C], F16, tag=f"conf{i}", name=f"conf_t{i}")
                for i in range(n_tiles)
            ]
            lt_t = [
                singles.tile([128, T, 9], F16, tag=f"lt{i}", name=f"lt_t{i}")
                for i in range(n_tiles)
            ]

            def tp(i):
                return 128 if i < n_tiles - 1 else 127

            for i in range(n_tiles):
                nc.gpsimd.dma_start(lt_t[i][: tp(i)], dram_ap(lt, 9, tp(i), i * T))
                nc.gpsimd.dma_start(conf_t[i][: tp(i)], dram_ap(conf, C, tp(i), i * T))

            for k in range(n_tiles):
                i = k
                P = tp(i)
                tlab = lt_t[i][:P, :, 8:9]  # [P,T,1] f16
                labpk = small.tile([128, T], F16, tag="labpk")
                nc.scalar.activation(labpk[:P], tlab.squeeze(), AF.Copy)

                # ---- conf path ----
                e_t = epool.tile([128, T, CP], F16, tag="e")
                nc.gpsimd.memset(e_t[:P, :, 81:82], 0.0)
                nc.scalar.activation(e_t[:P, :, 0:C], conf_t[i][:P], AF.Exp)
                s_t = small.tile([128, T], F16, tag="s")
                with nc.allow_low_precision("fp32-internal accum; fp16 write"):
                    fold_sum(nc, e_t, s_t, P)

                iseq = ipool.tile([128, T, CP], F16, tag="iseq")
                nc.vector.tensor_tensor(
                    iseq[:P],
                    iota16[:P, None, :].broadcast_to([P, T, CP]),
                    labpk[:P, :, None].broadcast_to([P, T, CP]),
                    OP.is_equal,
                )
                me = mpool.tile([128, T, CP], F16, tag="me")
                nc.gpsimd.memset(me[:P, :, 81:82], 0.0)
                nc.vector.tensor_tensor(
                    me[:P, :, 0:C], conf_t[i][:P], iseq[:P, :, 0:C], OP.mult
                )
                la = small.tile([128, T], F16, tag="la")
                with nc.allow_low_precision("one-hot row sum; fp16 write"):
                    fold_sum(nc, me, la, P)

                # per-anchor scalars [P, T]
                lns = small.tile([128, T], F16, tag="lns")
                nc.scalar.activation(lns[:P], s_t[:P], AF.Ln)
                nlp = small.tile([128, T], F16, tag="nlp")  # -log pt
                nc.vector.tensor_tensor(nlp[:P], lns[:P], la[:P], OP.subtract)
                pt = small.tile([128, T], F16, tag="pt")
                nc.scalar.activation(pt[:P], nlp[:P], AF.Exp, scale=-1.0)
                usq = small.tile([128, T], F16, tag="usq")  # (1-pt)^2
                nc.scalar.activation(usq[:P], pt[:P], AF.Square, bias=negone[:P])

                # ---- smooth-L1 per-anchor sum (mask applied by the one-hot) ----
                df = small.tile([128, T, 4], F16, tag="df")
                nc.vector.tensor_tensor(df[:P], lt_t[i][:P, :, 0:4], lt_t[i][:P, :, 4:8], OP.subtract)
                da = small.tile([128, T, 4], F16, tag="da")
                nc.scalar.activation(da[:P], df[:P], AF.Abs)
                mp = small.tile([128, T, 4], F16, tag="mp")
                nc.vector.tensor_scalar_min(mp[:P], da[:P], 1.0)
                u_t = small.tile([128, T, 4], F16, tag="u")
                nc.vector.tensor_tensor(u_t[:P], da[:P], mp[:P], OP.subtract)
                # sl1 = (da - mp) + 0.5*mp^2
                q2 = small.tile([128, T, 4], F16, tag="q2")
                nc.scalar.activation(q2[:P], mp[:P], AF.Square)
                sl = small.tile([128, T, 4], F16, tag="sl")
                nc.vector.scalar_tensor_tensor(sl[:P], q2[:P], 0.5, u_t[:P], OP.mult, OP.add)

                wv = small.tile([128, T, 3], F16, tag="wv")
                nc.gpsimd.memset(wv[:P, :, 1:2], 1.0)
                nc.vector.tensor_tensor(wv[:P, :, 0:1], usq[:P, :, None], nlp[:P, :, None], OP.mult)
                with nc.allow_low_precision("4-elem row sum; fp16 write"):
                    nc.vector.reduce_sum(wv[:P, :, 2:3], sl[:P], axis=AX.X)

                # ---- the grouped matmuls: h, cnt, loc-sum in one PSUM accum ----
                for g in range(n_full_g):
                    t0 = g * Q
                    nc.tensor.matmul(
                        ph6[:, :],
                        wv[:P, t0 : t0 + Q, :],
                        iseq[:P, t0 : t0 + Q, :],
                        start=(k == 0 and g == 0),
                        stop=(k == n_tiles - 1 and g == n_full_g - 1),
                    )
                if tail:
                    t0 = n_full_g * Q
                    nc.tensor.matmul(
                        ph3[:, :],
                        wv[:P, t0 : t0 + tail, :],
                        iseq[:P, t0 : t0 + tail, :],
                        start=(k == 0),
                        stop=(k == n_tiles - 1),
                    )

            # ---- finalize: ship raw PSUM blocks; host extracts diagonals ----
            hps = singles.tile([3 * Q, CP * Q], F32)
            nc.vector.tensor_copy(hps[:, :], ph6[:, :])
            nc.sync.dma_start(hist6[:, :], hps[:, :])
            if tail:
                hps3 = singles.tile([3 * tail, CP * tail], F32)
                nc.vector.tensor_copy(hps3[:, :], ph3[:, :])
                nc.sync.dma_start(hist3[:, :], hps3[:, :])

    nc.compile()
    return nc


_CACHED = {}


def _get_nc(A, APP, T):
    key = (A, APP, T)
    if key not in _CACHED:
        _CACHED[key] = build_kernel(A, APP, T)
    return _CACHED[key]


def extract_diag(blk, q):
    """blk: [ncores, 3q, 82q] grouped-matmul PSUM dump -> [ncores, 3, 81]
    by summing the q diagonal [3, 82] blocks (off-diagonals are garbage)."""
    nc_, _, _ = blk.shape
    out = np.zeros((nc_, 3, C), dtype=np.float64)
    for tq in range(q):
        out += blk[:, 3 * tq : 3 * tq + 3, CP * tq : CP * tq + C]
    return out


def combine_host(hists, alpha):
    """hists: [ncores, 3, 81] (rows: weighted, counts, sl1-sums); alpha: [81]."""
    h = hists[:, 0, :].sum(axis=0)
    cnt = hists[:, 1, :].sum(axis=0)
    alpha = alpha.astype(np.float64)
    denom = np.clip(alpha * cnt, 1.0, None)
    conf_loss = np.sum(alpha * h / denom)
    num_pos = cnt[1:].sum()
    loc_sum = hists[:, 2, 1:].sum()  # c>=1 selects positive anchors
    denom_loc = max(num_pos * 4.0, 1.0)
    loc_loss = loc_sum / denom_loc if num_pos > 0 else 0.0
    return np.float32(loc_loss), np.float32(conf_loss)


def kernel(loc_pred, conf_pred, targets, alpha, _trace=False):
    B, A, _ = conf_pred.shape
    assert B == 8 and A == 76725
    nc = _get_nc(A, 600, 75)
    conf16 = np.ascontiguousarray(conf_pred, dtype=np.float16)
    lt16 = np.empty((B, A, 9), dtype=np.float16)
    lt16[:, :, 0:4] = loc_pred
    lt16[:, :, 4:9] = targets
    in_maps = [{"conf": conf16[b], "lt": lt16[b]} for b in range(B)]
    res = run_bass_kernel_spmd(nc, in_maps, core_ids=list(range(B)), trace=_trace)
    h6 = np.stack([r["hist6"] for r in res.results]).astype(np.float64)
    h3 = np.stack([r["hist3"] for r in res.results]).astype(np.float64)
    hists = extract_diag(h6, 6) + extract_diag(h3, 3)
    out = combine_host(hists, np.asarray(alpha, dtype=np.float32))
    if _trace:
        return out, res
    return out
